# revision 44
# baseline (speedup 1.0000x reference)
"""Trainium2 Bass kernel for nn_Net_67765993996461.

Spiking CNN: conv2d -> LIF -> conv2d(dilated) -> LIF -> conv2d(dilated)
-> LIF -> time-mean -> FC.  Pure data parallel over batch: 32 images,
8 cores, 4 images/core.  Everything stays resident in SBUF per core.

Layout (per core, BL=4 local images; b' = 2*b2 + bh):
- scan space (ct/sp/v/u): partitions p = c + 64*bh, free = b2*T*M + t*M + m
  (b2-major so dup copies move long contiguous (t,m) runs).
- convs as K-packed matmuls: two t-taps stacked along K=128 (partition
  block g in the dup buffer holds the spike map shifted by -g*delta rows),
  batch-parity (bh) halves run as col-tiled concurrent matmuls
  (tile_position (0,0)/(0,64)).
- m-edge handling: no m-padding; each (tp,jj) tap matmul streams only its
  valid column range (jj emitted full-width-first so psum start covers
  every column).
- LIF scan: per step 2 DVE ops (u = a*v + c ; v = (u<1)*u) with the spike
  compare (s = u>=1) on the Pool engine off the critical path.  Layer 3
  accumulates sbar += s on Pool instead of storing a spike map.
- dup buffers S1D/S2D are persistent with pads zeroed once (Pool);
  interiors re-copied per layer in row-groups of 18, g=0 via Pool/SWDGE
  and g=1 via ACT/HWDGE, overlapped with the producing scan.
- FC folds the time-mean: y = (wf @ sbar)/129 + bf via a DRAM-bounce
  relayout and 32 K=80 accumulating matmuls.

Environment workarounds (this axon/fake_nrt runtime):
- walrus rejects multi-wait InstDrain -> split waits onto NOPs.
- branches hang -> merge all basic blocks into one (static code only).
- SP-engine DMAs with waits hang -> DMAs issued from ACT/DVE/Pool only.
"""
import sys

sys.path.insert(0, "/opt/trn_rl_repo")

import numpy as np
import ml_dtypes

import concourse.bass as bass
import concourse.mybir as mybir
from concourse import tile
from concourse.ap import AP
from concourse.bass_utils import run_bass_kernel_spmd

F32 = mybir.dt.float32
BF16 = mybir.dt.bfloat16
OP = mybir.AluOpType
AF = mybir.ActivationFunctionType

# ---------------- problem constants (hardcoded) ----------------
B, T0, M, C = 32, 128, 40, 64
NCORES = 8
BL = B // NCORES            # 4 images per core
T = T0 + 1                  # 129: conv1 output time length
TAU = np.float64(10.0) / 7.0
INV_TAU = float(1.0 / np.float32(TAU))
A_DECAY = float(np.float32(1.0) - np.float32(INV_TAU))   # 0.3

FS = 2 * M                  # 80 = (b2, m) scan-state row
TM = T * M                  # 5160
SL = 2 * TM                 # 10320 scan-space free size
I1F = BL * TM               # 20640 im2col free size

# conv2: rhs_dilation (4,3), padding (6,3): dt = 4i-6, dm = 3j-3
# conv3: rhs_dilation (16,9), padding (24,9): dt = 16i-24, dm = 9j-9
CONV2 = dict(dt0=(-6, 2), delta=4, dms=(-3, 0, 3), pt=6)
CONV3 = dict(dt0=(-24, 8), delta=16, dms=(-9, 0, 9), pt=24)
T1D = 137                   # rows r = pt+dt0+t span [0,137)
T2D = 161                   # rows span [0,161)
# Spike rows are stored 58 wide (9 zero pads each side of the 40 mel
# cols) in BOTH sp and the dup buffers, so dup copies are whole
# contiguous padded rows and conv rhs reads are full-width windows.
MDU, PMU = 58, 9
SPR = T * MDU               # 7482: one (b2) block of sp
SPF = 2 * SPR               # sp free size
BS1, BS2 = T1D * MDU, T2D * MDU
S1DF, S2DF = BL * BS1, BL * BS2

TCH = 6                     # conv chunk: t-rows per psum chunk
NCH = (T + TCH - 1) // TCH  # 22 chunks (last has 3 rows)
NTD = 18                    # dup row-group size (3 chunks)
JJO = (1, 0, 2)             # jj emission order: full-width tap first

# ---------------- runtime-environment patches ----------------
from concourse.tile import ScopedClock
import concourse.tile as _tile


def _patched_drain_and_barrier(self, tick_clock, wait_clock):
    carrier = self.nc.sync.nop(nofuse=True, hint="tail_drain_waits")
    wait_clock.add_sem_waits(
        carrier.ins, ScopedClock({None: tick_clock.global_clock})
    )
    waits = list(carrier.ins.sync_info.on_wait) if carrier.ins.sync_info else []
    if len(waits) > 1:
        carrier.ins.sync_info = mybir.SyncInfo(on_wait=[waits[0]], on_update=[])
        for w in waits[1:]:
            extra = self.nc.sync.nop(nofuse=True, hint="tail_drain_waits")
            extra.ins.sync_info = mybir.SyncInfo(on_wait=[w], on_update=[])
    self.nc.sync.drain()
    self.nc.all_engine_barrier()
    assert self.sems is not None
    popped = self.nc._tile_sem_poison_stack.pop()
    assert popped is self._sem_poison
    self.nc.clear_and_free_semaphores(list(self.sems.allocated().values()))
    self.nc.all_engine_barrier()


_tile.TileContext._drain_and_barrier = _patched_drain_and_barrier


def merge_bbs(nc):
    """Post-process for this runtime: (a) flatten the linear bb chain into
    one bb (branches hang), dropping UnconditionalBranch; (b) split
    instructions carrying more than one sem-wait — this walrus build
    rejects multi-wait sync setup — by hoisting extra waits onto NoOps
    emitted just before on the same engine."""
    import json

    wseq = [0]

    def split_waits(ins, out_list):
        si = ins.get("sync_info")
        waits = (si or {}).get("on_wait") or []
        if len(waits) > 1:
            for w in waits[:-1]:
                wseq[0] += 1
                out_list.append({
                    "debug": ins.get("debug", 0), "engine": ins["engine"],
                    "ins": [], "name": f"WN-{wseq[0]}", "opcode": "NoOp",
                    "outs": [],
                    "sync_info": {"on_update": [], "on_wait": [w]},
                })
            si["on_wait"] = [waits[-1]]
        out_list.append(ins)

    j = json.loads(mybir.module_to_json_string(nc.m))
    for fn in j["functions"]:
        blocks = fn["blocks"]
        merged = []
        for bi, blk in enumerate(blocks):
            nxt = blocks[bi + 1]["name"] if bi + 1 < len(blocks) else None
            for ins in blk["instructions"]:
                if ins.get("opcode") == "UnconditionalBranch":
                    assert nxt is not None and ins["target"] == nxt
                    continue
                split_waits(ins, merged)
        blocks[0]["instructions"] = merged
        fn["blocks"] = [blocks[0]]
    nc.m = mybir.module_from_json_string(json.dumps(j))
    return nc


# ---------------- device kernel ----------------
def build_nc(debug=False, reps=1):
    nc = bass.Bass("TRN2", target_bir_lowering=False, debug=False)

    x_d = nc.declare_dram_parameter("x", [T0, BL * M], BF16, isOutput=False)
    w1_d = nc.declare_dram_parameter("w1p", [12, 128], BF16, isOutput=False)
    w2_d = nc.declare_dram_parameter("w2p", [128, 768], BF16, isOutput=False)
    w3_d = nc.declare_dram_parameter("w3p", [128, 768], BF16, isOutput=False)
    fc_d = nc.declare_dram_parameter("fcp", [80, 32 * 12], BF16, isOutput=False)
    bf_d = nc.declare_dram_parameter("bf", [12], F32, isOutput=False)
    zz_d = nc.declare_dram_parameter("zz", [128], BF16, isOutput=False)
    y_d = nc.declare_dram_parameter("y", [BL, 12], F32, isOutput=True)
    if debug:
        dbg = {
            nm: nc.declare_dram_parameter(nm, [128, SL], BF16, isOutput=True)
            for nm in ("c1o", "c2o", "c3o")
        }
        for nm in ("s1o", "s2o"):
            dbg[nm] = nc.declare_dram_parameter(nm, [128, SPF], BF16,
                                                isOutput=True)
        dbg["sbo"] = nc.declare_dram_parameter("sbo", [128, FS], BF16,
                                               isOutput=True)

    with tile.TileContext(nc) as tc:
        with (
            tc.tile_pool(name="pool", bufs=1) as pool,
            tc.tile_pool(name="ppsum", bufs=4, space="PSUM") as ppsum,
            tc.tile_pool(name="pfc", bufs=1, space="PSUM") as pfc,
            tc.tile_pool(name="pdram", bufs=1, space="DRAM") as pdram,
        ):
            # ---- tiles ----
            # conv1 input I1M lives in S2D's space (S2D is only needed two
            # layers later): partitions p = 4*jjg + i hold 42-wide-row
            # padded copies of x, block jjg column-shifted by (2-jjg), so a
            # single K=12 matmul reading col 1+m sees x[t+i-2, m+jj-1] with
            # zero margins as m-padding.  I1M[p, b'*E42 + t*42 + c].
            S1D = pool.tile([128, S1DF], BF16)      # layer-1 spike dup buffer
            S2D = pool.tile([128, S2DF], BF16)      # layer-2 spike dup buffer
            ct = pool.tile([128, SL], BF16)         # conv out -> scan input
            sp = pool.tile([128, SPF], BF16)        # spike map, 58-wide rows
            v = pool.tile([128, FS], F32)
            ua = pool.tile([128, FS], F32)
            ub = pool.tile([128, FS], F32)
            u2 = [ua, ub]
            w1t = pool.tile([12, 128], BF16)        # conv1 w: p = 4*jj+i
            w2t = pool.tile([128, 768], BF16)
            w3t = pool.tile([128, 768], BF16)
            fct = pool.tile([80, 32 * 12], BF16)
            bft = pool.tile([12, 1], F32)
            sbar = pool.tile([128, FS], BF16)
            stmp = pool.tile([128, FS], BF16)
            fcr = pool.tile([80, 32 * BL], BF16)
            ysb = pool.tile([12, BL], F32)
            scr = pdram.tile([BL, C * M], BF16)

            # ---- preamble ----
            E42 = T * 42
            NB1 = BL * E42                # one jjg-block of I1M
            # I1M margins: zero cols 0 and 41 of every row, blocks 0..2
            # (Pool engine; partitions 0:12 is an aligned base)
            nc.gpsimd.memset(AP(S2D[:].tensor, S2D[:].offset,
                                [[S2DF, 12], [E42, BL], [42, T], [1, 1]]), 0.0)
            nc.gpsimd.memset(AP(S2D[:].tensor, S2D[:].offset + 41,
                                [[S2DF, 12], [E42, BL], [42, T], [1, 1]]), 0.0)
            # x loads into block jjg=1 (partitions 4..7), per (b', i); these
            # touch only rows [tlo,thi) cols [1,41) so they are disjoint
            # from all the zeroing -> wait-free -> safe on the SP queue.
            for i in range(4):
                tlo = max(0, 2 - i)
                thi = min(T, T0 + 2 - i)
                nt = thi - tlo
                for bp in range(BL):
                    src = AP(x_d.ap().tensor,
                             (tlo + i - 2) * (BL * M) + bp * M,
                             [[BL * M, nt], [1, M]])
                    dst = AP(S2D[:].tensor,
                             S2D[:].offset + (4 + i) * S2DF + bp * E42
                             + tlo * 42 + 1,
                             [[S2DF, 1], [42, nt], [1, M]])
                    nc.sync.dma_start(dst, src)
                # out-of-range t rows of block 1 (full 42-wide rows): zero
                # via zz DMAs on the Pool queue (they wait on the margin
                # memsets, which is fine off SP)
                base = S2D[:].offset + (4 + i) * S2DF
                for (o, n) in ((0, tlo * 42), (thi * 42, (T - thi) * 42)):
                    if n == 0:
                        continue
                    nc.gpsimd.dma_start(
                        AP(S2D[:].tensor, base + o,
                           [[S2DF, 1], [E42, BL], [1, n]]),
                        AP(zz_d.ap().tensor, 0, [[0, BL], [0, (n + 127) // 128],
                                                 [1, min(n, 128)]]))
            # column-shift copies: block0 = block1 << 1 col, block2 >> 1 col
            # (zero margins flow into the shifted m-pads)
            nc.scalar.dma_start(w1t[:], w1_d[:])
            nc.scalar.dma_start(
                AP(S2D[:].tensor, S2D[:].offset + 1, [[S2DF, 4], [1, NB1 - 1]]),
                AP(S2D[:].tensor, S2D[:].offset + 4 * S2DF,
                   [[S2DF, 4], [1, NB1 - 1]]))
            nc.scalar.dma_start(
                AP(S2D[:].tensor, S2D[:].offset + 8 * S2DF,
                   [[S2DF, 4], [1, NB1 - 1]]),
                AP(S2D[:].tensor, S2D[:].offset + 4 * S2DF + 1,
                   [[S2DF, 4], [1, NB1 - 1]]))
            nc.scalar.dma_start(w2t[:], w2_d[:])
            nc.scalar.dma_start(w3t[:], w3_d[:])
            nc.scalar.dma_start(fct[:], fc_d[:])
            nc.scalar.dma_start(bft[:], AP(bf_d.ap().tensor, 0, [[1, 12], [1, 1]]))

            # ---- preamble: one-time zeroing on Pool ----
            nc.gpsimd.memset(sbar[:], 0.0)
            # sp m-pad strips (cols [0,9) and [49,58) of every row)
            nc.gpsimd.memset(AP(sp[:].tensor, sp[:].offset,
                                [[SPF, 128], [SPR, 2], [MDU, T], [1, PMU]]), 0.0)
            nc.gpsimd.memset(AP(sp[:].tensor, sp[:].offset + PMU + M,
                                [[SPF, 128], [SPR, 2], [MDU, T], [1, PMU]]), 0.0)
            # S1D pad rows (outside the per-g interior [pt-g*delta, +T));
            # m-pads inside the interior arrive with the dup rows (zero in
            # sp).  S2D's pad rows are deferred to after layer 1 because
            # its space currently holds I1M.
            def sd_pad_rows(SD, SDF, BSx, TD, geom):
                for g in range(2):
                    lo = geom["pt"] - g * geom["delta"]      # interior start row
                    hi = lo + T                              # interior end row
                    base = SD[:].offset + g * 64 * SDF
                    if lo > 0:
                        nc.gpsimd.memset(
                            AP(SD[:].tensor, base,
                               [[SDF, 64], [BSx, BL], [1, lo * MDU]]), 0.0)
                    if hi < TD:
                        nc.gpsimd.memset(
                            AP(SD[:].tensor, base + hi * MDU,
                               [[SDF, 64], [BSx, BL], [1, (TD - hi) * MDU]]), 0.0)

            sd_pad_rows(S1D, S1DF, BS1, T1D, CONV2)

            # ---- helpers ----
            def conv_chunk(layer, ch):
                """Emit matmuls + drain for chunk ch of layer. Returns ct rows."""
                u0 = ch * TCH
                tc_ = min(TCH, T - u0)
                ncols = 2 * tc_ * M
                pc = ppsum.tile([128, 2 * TCH * M], F32, tag="pc")
                if layer == 1:
                    # single K=12 matmul per bh half over I1M (in S2D space):
                    # K-row p = 4*jjg+i reads x[t+i-2, m+jj-1] at col 1+m
                    for bh in range(2):
                        rhs = AP(S2D[:].tensor,
                                 S2D[:].offset + bh * E42 + u0 * 42 + 1,
                                 [[S2DF, 12], [2 * E42, 2], [42, tc_], [1, M]])
                        nc.tensor.matmul(
                            pc[bh * 64:(bh + 1) * 64, 0:ncols],
                            w1t[0:12, bh * 64:(bh + 1) * 64],
                            rhs, start=True, stop=True,
                            tile_position=(0, bh * 64))
                else:
                    geom = CONV2 if layer == 2 else CONV3
                    SD, SDF, BSx = (S1D, S1DF, BS1) if layer == 2 else (S2D, S2DF, BS2)
                    wt = w2t if layer == 2 else w3t
                    for mmo in range(6):
                        tp, jj = mmo // 3, JJO[mmo % 3]
                        dm = geom["dms"][jj]
                        r0 = geom["pt"] + geom["dt0"][tp] + u0
                        for bh in range(2):
                            rhs = AP(SD[:].tensor,
                                     SD[:].offset + bh * BSx + r0 * MDU
                                     + (PMU + dm),
                                     [[SDF, 128], [2 * BSx, 2], [MDU, tc_],
                                      [1, M]])
                            nc.tensor.matmul(
                                pc[bh * 64:(bh + 1) * 64, 0:ncols],
                                wt[0:128, mmo * 128 + bh * 64:
                                   mmo * 128 + bh * 64 + 64],
                                rhs, start=(mmo == 0), stop=(mmo == 5),
                                tile_position=(0, bh * 64))
                # drain psum -> ct rows u0..u0+tc_ (scan space, b2-major)
                src = AP(pc[:].tensor, pc[:].offset,
                         [[2 * TCH * M, 128], [tc_ * M, 2], [M, tc_], [1, M]])
                dst = AP(ct[:].tensor, ct[:].offset + u0 * M,
                         [[SL, 128], [TM, 2], [M, tc_], [1, M]])
                nc.scalar.activation(dst, src, AF.Copy, scale=1.0)
                return u0, tc_

            def scan_rows(layer, u0, tc_):
                for t in range(u0, u0 + tc_):
                    ud = u2[t % 2]
                    cs = AP(ct[:].tensor, ct[:].offset + t * M,
                            [[SL, 128], [TM, 2], [1, M]])
                    nc.vector.scalar_tensor_tensor(
                        out=ud[:], in0=v[:], scalar=A_DECAY, in1=cs,
                        op0=OP.mult, op1=OP.add)
                    nc.vector.scalar_tensor_tensor(
                        out=v[:], in0=ud[:], scalar=1.0, in1=ud[:],
                        op0=OP.is_lt, op1=OP.mult)
                    if layer < 3 or debug:
                        ssl = AP(sp[:].tensor, sp[:].offset + t * MDU + PMU,
                                 [[SPF, 128], [SPR, 2], [1, M]])
                        nc.gpsimd.tensor_scalar(
                            out=ssl, in0=ud[:], scalar1=1.0, scalar2=None,
                            op0=OP.is_ge)
                    if layer == 3:
                        if debug:
                            nc.gpsimd.tensor_copy(stmp[:], ssl)
                        else:
                            nc.gpsimd.tensor_scalar(
                                out=stmp[:], in0=ud[:], scalar1=1.0,
                                scalar2=None, op0=OP.is_ge)
                        nc.gpsimd.tensor_add(out=sbar[:], in0=sbar[:],
                                             in1=stmp[:])

            def dup_group(layer, k):
                """Copy spike rows [18k, 18k+nt) into the dup buffer, both
                g-shifted copies. g=0 via Pool queue, g=1 via ACT queue."""
                geom = CONV2 if layer == 1 else CONV3
                SD, SDF, BSx = (S1D, S1DF, BS1) if layer == 1 else (S2D, S2DF, BS2)
                t0 = NTD * k
                nt = min(NTD, T - t0)
                for g in range(2):
                    eng = nc.scalar
                    for bh in range(2):
                        src = AP(sp[:].tensor,
                                 sp[:].offset + bh * 64 * SPF + t0 * MDU,
                                 [[SPF, 64], [SPR, 2], [1, nt * MDU]])
                        dst = AP(SD[:].tensor,
                                 SD[:].offset + g * 64 * SDF + bh * BSx
                                 + (geom["pt"] - g * geom["delta"] + t0) * MDU,
                                 [[SDF, 64], [2 * BSx, 2], [1, nt * MDU]])
                        eng.dma_start(dst, src)

            def dump(name_s, name_c, layer):
                if not debug:
                    return
                if name_s:
                    nc.scalar.dma_start(dbg[name_s].ap(), sp[:])
                nc.scalar.dma_start(dbg[name_c].ap(), ct[:])

            # ================= emission =================
            for _rep in range(reps):
                if _rep > 0:
                    nc.gpsimd.memset(sbar[:], 0.0)
                for layer in (1, 2, 3):
                    nc.vector.memset(v[:], 0.0)
                    for ch in range(NCH):
                        u0, tc_ = conv_chunk(layer, ch)
                        scan_rows(layer, u0, tc_)
                        if layer < 3 and (ch % 3 == 2 or ch == NCH - 1):
                            dup_group(layer, ch // 3)
                    if layer == 1:
                        if _rep == 0:
                            # I1M (conv1 input) is done with S2D's space now
                            sd_pad_rows(S2D, S2DF, BS2, T2D, CONV3)
                        dump("s1o", "c1o", 1)
                    elif layer == 2:
                        dump("s2o", "c2o", 2)
                    else:
                        dump(None, "c3o", 3)

            if debug:
                nc.scalar.dma_start(dbg["sbo"].ap(), sbar[:])

            # ---- FC: y = (wf @ sbar)/T + bf ----
            # sbar [(c,bh), (b2,m)] -> scr[b' , c*40+m]  (b' = 2*b2 + bh)
            for bh in range(2):
                src = AP(sbar[:].tensor, sbar[:].offset + bh * 64 * FS,
                         [[FS, 64], [M, 2], [1, M]])
                dst = AP(scr[:].tensor, scr[:].offset + bh * C * M,
                         [[M, 64], [2 * C * M, 2], [1, M]])
                nc.scalar.dma_start(dst, src)
            with nc.allow_non_contiguous_dma(reason="tiny fc relayout"):
                for b in range(BL):
                    nc.scalar.dma_start(
                        AP(fcr[:].tensor, fcr[:].offset + b * 32,
                           [[32 * BL, 80], [1, 32]]),
                        AP(scr[:].tensor, scr[:].offset + b * C * M,
                           [[1, 80], [80, 32]]))
            pf = pfc.tile([12, BL], F32)
            for k in range(32):
                nc.tensor.matmul(
                    pf[:, :], fct[0:80, k * 12:(k + 1) * 12],
                    AP(fcr[:].tensor, fcr[:].offset + k, [[32 * BL, 80], [32, BL]]),
                    start=(k == 0), stop=(k == 31))
            nc.scalar.activation(ysb[:], pf[:, :], AF.Identity,
                                 bias=bft[:, 0:1], scale=float(1.0 / T))
            nc.scalar.dma_start(AP(y_d.ap().tensor, 0, [[1, 12], [12, BL]]), ysb[:])

    return nc


# ---------------- host-side weight packing ----------------
def pack_inputs(x, w1, w2, w3, wf, bf):
    """Returns list of per-core input maps (bf16-packed)."""
    inv_tau = np.float32(INV_TAU)
    x = np.asarray(x, np.float32)
    w1 = np.asarray(w1, np.float32)
    w2 = np.asarray(w2, np.float32)
    w3 = np.asarray(w3, np.float32)
    wf = np.asarray(wf, np.float32)

    w1p = np.zeros((12, 128), np.float32)
    for i in range(4):
        for jj in range(3):
            w1p[4 * jj + i, 0:64] = w1[:, 0, i, jj] * inv_tau
    w1p[:, 64:128] = w1p[:, 0:64]
    w1p = w1p.astype(ml_dtypes.bfloat16)

    def pack_w(w):
        wp = np.zeros((128, 768), np.float32)
        for mmo in range(6):
            tp, jj = mmo // 3, JJO[mmo % 3]
            for g in range(2):
                i = 2 * tp + g
                blk = w[:, :, i, jj].T * inv_tau   # [c_in, c_out]
                wp[g * 64:(g + 1) * 64, mmo * 128:mmo * 128 + 64] = blk
                wp[g * 64:(g + 1) * 64, mmo * 128 + 64:mmo * 128 + 128] = blk
        return wp.astype(ml_dtypes.bfloat16)

    w2p = pack_w(w2)
    w3p = pack_w(w3)
    fcp = np.zeros((80, 32 * 12), np.float32)
    for k in range(32):
        fcp[:, k * 12:(k + 1) * 12] = wf[:, 80 * k:80 * (k + 1)].T
    fcp = fcp.astype(ml_dtypes.bfloat16)
    bfv = np.asarray(bf, np.float32).reshape(12)

    maps = []
    for c in range(NCORES):
        xc = x[c * BL:(c + 1) * BL, 0]                    # [BL, T0, M]
        xp = np.ascontiguousarray(
            xc.transpose(1, 0, 2).reshape(T0, BL * M)).astype(ml_dtypes.bfloat16)
        maps.append({
            "x": xp, "w1p": w1p, "w2p": w2p, "w3p": w3p, "fcp": fcp, "bf": bfv,
            "zz": np.zeros(128, ml_dtypes.bfloat16),
        })
    return maps


_CACHED = {}


def get_nc(debug=False, reps=1):
    key = (bool(debug), reps)
    if key not in _CACHED:
        nc = build_nc(debug=debug, reps=reps)
        merge_bbs(nc)
        _CACHED[key] = nc
    return _CACHED[key]


def make_runner(nc, in_maps):
    """Build the sharded PJRT callable once (mimics bass2jax.run_bass_via_pjrt)
    so repeated calls reuse the compiled executable for timing."""
    import jax
    from jax.sharding import Mesh, PartitionSpec
    from jax.experimental.shard_map import shard_map
    from concourse import bass2jax
    from concourse.bass2jax import _bass_exec_p, install_neuronx_cc_hook, partition_id_tensor

    install_neuronx_cc_hook()
    n_cores = len(in_maps)
    partition_name = nc.partition_id_tensor.name if nc.partition_id_tensor else None
    in_names, out_names, out_avals, zero_outs = [], [], [], []
    for alloc in nc.m.functions[0].allocations:
        if not isinstance(alloc, mybir.MemoryLocationSet):
            continue
        name = alloc.memorylocations[0].name
        if alloc.kind == "ExternalInput":
            if name != partition_name:
                in_names.append(name)
        elif alloc.kind == "ExternalOutput":
            out_names.append(name)
            shape = tuple(alloc.tensor_shape)
            dtype = mybir.dt.np(alloc.dtype)
            out_avals.append(jax.core.ShapedArray(shape, dtype))
            zero_outs.append(np.zeros(shape, dtype))
    n_params = len(in_names)
    n_outs = len(out_avals)
    in_names_all = in_names + out_names + ([partition_name] if partition_name else [])

    def _body(*args):
        operands = list(args)
        if partition_name is not None:
            operands.append(partition_id_tensor())
        outs = _bass_exec_p.bind(
            *operands,
            out_avals=tuple(out_avals),
            in_names=tuple(in_names_all),
            out_names=tuple(out_names),
            lowering_input_output_aliases=(),
            sim_require_finite=True,
            sim_require_nnan=True,
            nc=nc,
        )
        return tuple(outs)

    devices = jax.devices()[:n_cores]
    mesh = Mesh(np.asarray(devices), ("core",))
    donate = tuple(range(n_params, n_params + n_outs))
    sharded = jax.jit(
        shard_map(_body, mesh=mesh,
                  in_specs=(PartitionSpec("core"),) * (n_params + n_outs),
                  out_specs=(PartitionSpec("core"),) * n_outs,
                  check_rep=False),
        donate_argnums=donate, keep_unused=True)
    concat_in = [
        np.concatenate([np.asarray(in_maps[c][nm]) for c in range(n_cores)], axis=0)
        for nm in in_names
    ]

    def run():
        zeros = [np.zeros((n_cores * z.shape[0], *z.shape[1:]), z.dtype)
                 for z in zero_outs]
        out_arrs = sharded(*concat_in, *zeros)
        out_arrs = [np.asarray(a) for a in out_arrs]
        return [
            {nm: out_arrs[i].reshape(n_cores, *out_avals[i].shape)[c]
             for i, nm in enumerate(out_names)}
            for c in range(n_cores)
        ]

    return run


def kernel(x, w1, w2, w3, wf, bf):
    nc = get_nc(debug=False)
    in_maps = pack_inputs(np.asarray(x), np.asarray(w1), np.asarray(w2),
                          np.asarray(w3), np.asarray(wf), np.asarray(bf))
    res = run_bass_kernel_spmd(nc, in_maps, list(range(NCORES)))
    y = np.concatenate([res.results[c]["y"] for c in range(NCORES)], axis=0)
    return y.astype(np.float32)


# revision 53
# speedup vs baseline: 6.3010x; 6.3010x over previous
"""Trainium2 Bass kernel for nn_Net_67765993996461.

Spiking CNN: conv2d -> LIF -> conv2d(dilated) -> LIF -> conv2d(dilated)
-> LIF -> time-mean -> FC.  Pure data parallel over batch: 32 images,
8 cores, 4 images/core.  Everything stays resident in SBUF per core.

Layout (per core, BL=4 local images; b' = 2*b2 + bh):
- scan space (ct/sp/v/u): partitions p = c + 64*bh, free = b2*T*M + t*M + m
  (b2-major so dup copies move long contiguous (t,m) runs).
- convs as K-packed matmuls: two t-taps stacked along K=128 (partition
  block g in the dup buffer holds the spike map shifted by -g*delta rows),
  batch-parity (bh) halves run as col-tiled concurrent matmuls
  (tile_position (0,0)/(0,64)).
- m-edge handling: no m-padding; each (tp,jj) tap matmul streams only its
  valid column range (jj emitted full-width-first so psum start covers
  every column).
- LIF scan: per step 2 DVE ops (u = a*v + c ; v = (u<1)*u) with the spike
  compare (s = u>=1) on the Pool engine off the critical path.  Layer 3
  accumulates sbar += s on Pool instead of storing a spike map.
- dup buffers S1D/S2D are persistent with pads zeroed once (Pool);
  interiors re-copied per layer in row-groups of 18, g=0 via Pool/SWDGE
  and g=1 via ACT/HWDGE, overlapped with the producing scan.
- FC folds the time-mean: y = (wf @ sbar)/129 + bf via a DRAM-bounce
  relayout and 32 K=80 accumulating matmuls.

Environment workarounds (this axon/fake_nrt runtime):
- walrus rejects multi-wait InstDrain -> split waits onto NOPs.
- branches hang -> merge all basic blocks into one (static code only).
- SP-engine DMAs with waits hang -> DMAs issued from ACT/DVE/Pool only.
"""
import sys

sys.path.insert(0, "/opt/trn_rl_repo")

import numpy as np
import ml_dtypes

import concourse.bass as bass
import concourse.mybir as mybir
from concourse import tile
from concourse.ap import AP
from concourse.bass_utils import run_bass_kernel_spmd

F32 = mybir.dt.float32
BF16 = mybir.dt.bfloat16
OP = mybir.AluOpType
AF = mybir.ActivationFunctionType

# ---------------- problem constants (hardcoded) ----------------
B, T0, M, C = 32, 128, 40, 64
NCORES = 8
BL = B // NCORES            # 4 images per core
T = T0 + 1                  # 129: conv1 output time length
TAU = np.float64(10.0) / 7.0
INV_TAU = float(1.0 / np.float32(TAU))
A_DECAY = float(np.float32(1.0) - np.float32(INV_TAU))   # 0.3

FS = 2 * M                  # 80 = (b2, m) scan-state row
TM = T * M                  # 5160
SL = 2 * TM                 # 10320 scan-space free size
I1F = BL * TM               # 20640 im2col free size

# conv2: rhs_dilation (4,3), padding (6,3): dt = 4i-6, dm = 3j-3
# conv3: rhs_dilation (16,9), padding (24,9): dt = 16i-24, dm = 9j-9
CONV2 = dict(dt0=(-6, 2), delta=4, dms=(-3, 0, 3), pt=6)
CONV3 = dict(dt0=(-24, 8), delta=16, dms=(-9, 0, 9), pt=24)
T1D = 137                   # rows r = pt+dt0+t span [0,137)
T2D = 161                   # rows span [0,161)
# Spike rows are stored 58 wide (9 zero pads each side of the 40 mel
# cols) in BOTH sp and the dup buffers, so dup copies are whole
# contiguous padded rows and conv rhs reads are full-width windows.
MDU, PMU = 58, 9
SPR = T * MDU               # 7482: one (b2) block of sp
SPF = 2 * SPR               # sp free size
BS1, BS2 = T1D * MDU, T2D * MDU
S1DF, S2DF = BL * BS1, BL * BS2

TCH = 6                     # conv chunk: t-rows per psum chunk
NCH = (T + TCH - 1) // TCH  # 22 chunks (last has 3 rows)
NTD = 18                    # dup row-group size (3 chunks)
JJO = (1, 0, 2)             # jj emission order: full-width tap first

# ---------------- runtime-environment patches ----------------
from concourse.tile import ScopedClock
import concourse.tile as _tile


def _patched_drain_and_barrier(self, tick_clock, wait_clock):
    carrier = self.nc.sync.nop(nofuse=True, hint="tail_drain_waits")
    wait_clock.add_sem_waits(
        carrier.ins, ScopedClock({None: tick_clock.global_clock})
    )
    waits = list(carrier.ins.sync_info.on_wait) if carrier.ins.sync_info else []
    if len(waits) > 1:
        carrier.ins.sync_info = mybir.SyncInfo(on_wait=[waits[0]], on_update=[])
        for w in waits[1:]:
            extra = self.nc.sync.nop(nofuse=True, hint="tail_drain_waits")
            extra.ins.sync_info = mybir.SyncInfo(on_wait=[w], on_update=[])
    self.nc.sync.drain()
    self.nc.all_engine_barrier()
    assert self.sems is not None
    popped = self.nc._tile_sem_poison_stack.pop()
    assert popped is self._sem_poison
    self.nc.clear_and_free_semaphores(list(self.sems.allocated().values()))
    self.nc.all_engine_barrier()


_tile.TileContext._drain_and_barrier = _patched_drain_and_barrier


def merge_bbs(nc):
    """Post-process for this runtime: (a) flatten the linear bb chain into
    one bb (branches hang), dropping UnconditionalBranch; (b) split
    instructions carrying more than one sem-wait — this walrus build
    rejects multi-wait sync setup — by hoisting extra waits onto NoOps
    emitted just before on the same engine."""
    import json

    wseq = [0]

    def split_waits(ins, out_list):
        si = ins.get("sync_info")
        waits = (si or {}).get("on_wait") or []
        if len(waits) > 1:
            for w in waits[:-1]:
                wseq[0] += 1
                out_list.append({
                    "debug": ins.get("debug", 0), "engine": ins["engine"],
                    "ins": [], "name": f"WN-{wseq[0]}", "opcode": "NoOp",
                    "outs": [],
                    "sync_info": {"on_update": [], "on_wait": [w]},
                })
            si["on_wait"] = [waits[-1]]
        out_list.append(ins)

    j = json.loads(mybir.module_to_json_string(nc.m))
    for fn in j["functions"]:
        blocks = fn["blocks"]
        merged = []
        for bi, blk in enumerate(blocks):
            nxt = blocks[bi + 1]["name"] if bi + 1 < len(blocks) else None
            for ins in blk["instructions"]:
                if ins.get("opcode") == "UnconditionalBranch":
                    assert nxt is not None and ins["target"] == nxt
                    continue
                split_waits(ins, merged)
        blocks[0]["instructions"] = merged
        fn["blocks"] = [blocks[0]]
    nc.m = mybir.module_from_json_string(json.dumps(j))
    return nc


# ---------------- device kernel ----------------
def build_nc(debug=False, reps=1, spike_pool=False):
    nc = bass.Bass("TRN2", target_bir_lowering=False, debug=False)

    x_d = nc.declare_dram_parameter("x", [T0, BL * M], BF16, isOutput=False)
    w1_d = nc.declare_dram_parameter("w1p", [12, 128], BF16, isOutput=False)
    w2_d = nc.declare_dram_parameter("w2p", [128, 768], BF16, isOutput=False)
    w3_d = nc.declare_dram_parameter("w3p", [128, 768], BF16, isOutput=False)
    fc_d = nc.declare_dram_parameter("fcp", [80, 32 * 12], BF16, isOutput=False)
    bf_d = nc.declare_dram_parameter("bf", [12], F32, isOutput=False)
    zz_d = nc.declare_dram_parameter("zz", [128], BF16, isOutput=False)
    y_d = nc.declare_dram_parameter("y", [BL, 12], F32, isOutput=True)
    if debug:
        dbg = {
            nm: nc.declare_dram_parameter(nm, [128, SL], BF16, isOutput=True)
            for nm in ("c1o", "c2o", "c3o")
        }
        for nm in ("s1o", "s2o"):
            dbg[nm] = nc.declare_dram_parameter(nm, [128, SPF], BF16,
                                                isOutput=True)
        dbg["sbo"] = nc.declare_dram_parameter("sbo", [128, FS], BF16,
                                               isOutput=True)

    with tile.TileContext(nc) as tc:
        with (
            tc.tile_pool(name="pool", bufs=1) as pool,
            tc.tile_pool(name="ppsum", bufs=4, space="PSUM") as ppsum,
            tc.tile_pool(name="pfc", bufs=1, space="PSUM") as pfc,
            tc.tile_pool(name="pdram", bufs=1, space="DRAM") as pdram,
        ):
            # ---- tiles ----
            # conv1 input I1M lives in S2D's space (S2D is only needed two
            # layers later): partitions p = 4*jjg + i hold 42-wide-row
            # padded copies of x, block jjg column-shifted by (2-jjg), so a
            # single K=12 matmul reading col 1+m sees x[t+i-2, m+jj-1] with
            # zero margins as m-padding.  I1M[p, b'*E42 + t*42 + c].
            S1D = pool.tile([128, S1DF], BF16)      # layer-1 spike dup buffer
            S2D = pool.tile([128, S2DF], BF16)      # layer-2 spike dup buffer
            ct = pool.tile([128, SL], BF16)         # conv out -> scan input
            sp = pool.tile([128, SPF], BF16)        # spike map, 58-wide rows
            v = pool.tile([128, FS], F32)
            ua = pool.tile([128, FS], F32)
            ub = pool.tile([128, FS], F32)
            u2 = [ua, ub]
            w1t = pool.tile([12, 128], BF16)        # conv1 w: p = 4*jj+i
            w2t = pool.tile([128, 768], BF16)
            w3t = pool.tile([128, 768], BF16)
            fct = pool.tile([80, 32 * 12], BF16)
            bft = pool.tile([12, 1], F32)
            sbar = pool.tile([128, FS], BF16)
            fcr = pool.tile([80, 32 * BL], BF16)
            ysb = pool.tile([12, BL], F32)
            scr = pdram.tile([BL, C * M], BF16)

            # ---- preamble ----
            E42 = T * 42
            NB1 = BL * E42                # one jjg-block of I1M
            # I1M margins: zero cols 0 and 41 of every row, blocks 0..2
            # (Pool engine; partitions 0:12 is an aligned base)
            nc.gpsimd.memset(AP(S2D[:].tensor, S2D[:].offset,
                                [[S2DF, 12], [E42, BL], [42, T], [1, 1]]), 0.0)
            nc.gpsimd.memset(AP(S2D[:].tensor, S2D[:].offset + 41,
                                [[S2DF, 12], [E42, BL], [42, T], [1, 1]]), 0.0)
            # x loads into block jjg=1 (partitions 4..7), per (b', i); these
            # touch only rows [tlo,thi) cols [1,41) so they are disjoint
            # from all the zeroing -> wait-free -> safe on the SP queue.
            for i in range(4):
                tlo = max(0, 2 - i)
                thi = min(T, T0 + 2 - i)
                nt = thi - tlo
                for bp in range(BL):
                    src = AP(x_d.ap().tensor,
                             (tlo + i - 2) * (BL * M) + bp * M,
                             [[BL * M, nt], [1, M]])
                    dst = AP(S2D[:].tensor,
                             S2D[:].offset + (4 + i) * S2DF + bp * E42
                             + tlo * 42 + 1,
                             [[S2DF, 1], [42, nt], [1, M]])
                    nc.sync.dma_start(dst, src)
                # out-of-range t rows of block 1 (full 42-wide rows): zero
                # via zz DMAs on the Pool queue (they wait on the margin
                # memsets, which is fine off SP)
                base = S2D[:].offset + (4 + i) * S2DF
                for (o, n) in ((0, tlo * 42), (thi * 42, (T - thi) * 42)):
                    if n == 0:
                        continue
                    nc.gpsimd.dma_start(
                        AP(S2D[:].tensor, base + o,
                           [[S2DF, 1], [E42, BL], [1, n]]),
                        AP(zz_d.ap().tensor, 0, [[0, BL], [0, (n + 127) // 128],
                                                 [1, min(n, 128)]]))
            # column-shift copies: block0 = block1 << 1 col, block2 >> 1 col
            # (zero margins flow into the shifted m-pads)
            nc.scalar.dma_start(w1t[:], w1_d[:])
            nc.scalar.dma_start(
                AP(S2D[:].tensor, S2D[:].offset + 1, [[S2DF, 4], [1, NB1 - 1]]),
                AP(S2D[:].tensor, S2D[:].offset + 4 * S2DF,
                   [[S2DF, 4], [1, NB1 - 1]]))
            nc.scalar.dma_start(
                AP(S2D[:].tensor, S2D[:].offset + 8 * S2DF,
                   [[S2DF, 4], [1, NB1 - 1]]),
                AP(S2D[:].tensor, S2D[:].offset + 4 * S2DF + 1,
                   [[S2DF, 4], [1, NB1 - 1]]))
            nc.scalar.dma_start(w2t[:], w2_d[:])
            nc.scalar.dma_start(w3t[:], w3_d[:])
            nc.scalar.dma_start(fct[:], fc_d[:])
            nc.scalar.dma_start(bft[:], AP(bf_d.ap().tensor, 0, [[1, 12], [1, 1]]))

            # ---- preamble: one-time zeroing on Pool ----
            # sp m-pad strips (cols [0,9) and [49,58) of every row)
            nc.gpsimd.memset(AP(sp[:].tensor, sp[:].offset,
                                [[SPF, 128], [SPR, 2], [MDU, T], [1, PMU]]), 0.0)
            nc.gpsimd.memset(AP(sp[:].tensor, sp[:].offset + PMU + M,
                                [[SPF, 128], [SPR, 2], [MDU, T], [1, PMU]]), 0.0)
            # S1D pad rows (outside the per-g interior [pt-g*delta, +T));
            # m-pads inside the interior arrive with the dup rows (zero in
            # sp).  S2D's pad rows are deferred to after layer 1 because
            # its space currently holds I1M.
            def sd_pad_rows(SD, SDF, BSx, TD, geom):
                for g in range(2):
                    lo = geom["pt"] - g * geom["delta"]      # interior start row
                    hi = lo + T                              # interior end row
                    base = SD[:].offset + g * 64 * SDF
                    if lo > 0:
                        nc.gpsimd.memset(
                            AP(SD[:].tensor, base,
                               [[SDF, 64], [BSx, BL], [1, lo * MDU]]), 0.0)
                    if hi < TD:
                        nc.gpsimd.memset(
                            AP(SD[:].tensor, base + hi * MDU,
                               [[SDF, 64], [BSx, BL], [1, (TD - hi) * MDU]]), 0.0)

            sd_pad_rows(S1D, S1DF, BS1, T1D, CONV2)

            # ---- helpers ----
            def conv_chunk(layer, ch):
                """Emit matmuls + drain for chunk ch of layer. Returns ct rows."""
                u0 = ch * TCH
                tc_ = min(TCH, T - u0)
                ncols = 2 * tc_ * M
                pc = ppsum.tile([128, 2 * TCH * M], F32, tag="pc")
                if layer == 1:
                    # single K=12 matmul per bh half over I1M (in S2D space):
                    # K-row p = 4*jjg+i reads x[t+i-2, m+jj-1] at col 1+m
                    for bh in range(2):
                        rhs = AP(S2D[:].tensor,
                                 S2D[:].offset + bh * E42 + u0 * 42 + 1,
                                 [[S2DF, 12], [2 * E42, 2], [42, tc_], [1, M]])
                        nc.tensor.matmul(
                            pc[bh * 64:(bh + 1) * 64, 0:ncols],
                            w1t[0:12, bh * 64:(bh + 1) * 64],
                            rhs, start=True, stop=True,
                            tile_position=(0, bh * 64))
                else:
                    geom = CONV2 if layer == 2 else CONV3
                    SD, SDF, BSx = (S1D, S1DF, BS1) if layer == 2 else (S2D, S2DF, BS2)
                    wt = w2t if layer == 2 else w3t
                    for mmo in range(6):
                        tp, jj = mmo // 3, JJO[mmo % 3]
                        dm = geom["dms"][jj]
                        r0 = geom["pt"] + geom["dt0"][tp] + u0
                        for bh in range(2):
                            rhs = AP(SD[:].tensor,
                                     SD[:].offset + bh * BSx + r0 * MDU
                                     + (PMU + dm),
                                     [[SDF, 128], [2 * BSx, 2], [MDU, tc_],
                                      [1, M]])
                            nc.tensor.matmul(
                                pc[bh * 64:(bh + 1) * 64, 0:ncols],
                                wt[0:128, mmo * 128 + bh * 64:
                                   mmo * 128 + bh * 64 + 64],
                                rhs, start=(mmo == 0), stop=(mmo == 5),
                                tile_position=(0, bh * 64))
                # drain psum -> ct rows u0..u0+tc_ (scan space, b2-major)
                src = AP(pc[:].tensor, pc[:].offset,
                         [[2 * TCH * M, 128], [tc_ * M, 2], [M, tc_], [1, M]])
                dst = AP(ct[:].tensor, ct[:].offset + u0 * M,
                         [[SL, 128], [TM, 2], [M, tc_], [1, M]])
                nc.scalar.activation(dst, src, AF.Copy, scale=1.0)
                return u0, tc_

            def scan_rows(layer, u0, tc_):
                for t in range(u0, u0 + tc_):
                    ud = u2[t % 2]
                    cs = AP(ct[:].tensor, ct[:].offset + t * M,
                            [[SL, 128], [TM, 2], [1, M]])
                    ssl = AP(sp[:].tensor, sp[:].offset + t * MDU + PMU,
                             [[SPF, 128], [SPR, 2], [1, M]])
                    nc.vector.scalar_tensor_tensor(
                        out=ud[:], in0=v[:], scalar=A_DECAY, in1=cs,
                        op0=OP.mult, op1=OP.add)
                    seng = nc.gpsimd if spike_pool else nc.vector
                    seng.tensor_scalar(
                        out=ssl, in0=ud[:], scalar1=1.0, scalar2=None,
                        op0=OP.is_ge)
                    nc.vector.scalar_tensor_tensor(
                        out=v[:], in0=ud[:], scalar=1.0, in1=ud[:],
                        op0=OP.is_lt, op1=OP.mult)

            def dup_group(layer, k):
                """Copy spike rows [18k, 18k+nt) into the dup buffer, both
                g-shifted copies, on the Pool/SWDGE queue."""
                geom = CONV2 if layer == 1 else CONV3
                SD, SDF, BSx = (S1D, S1DF, BS1) if layer == 1 else (S2D, S2DF, BS2)
                t0 = NTD * k
                nt = min(NTD, T - t0)
                for g in range(2):
                    eng = nc.gpsimd
                    for bh in range(2):
                        src = AP(sp[:].tensor,
                                 sp[:].offset + bh * 64 * SPF + t0 * MDU,
                                 [[SPF, 64], [SPR, 2], [1, nt * MDU]])
                        dst = AP(SD[:].tensor,
                                 SD[:].offset + g * 64 * SDF + bh * BSx
                                 + (geom["pt"] - g * geom["delta"] + t0) * MDU,
                                 [[SDF, 64], [2 * BSx, 2], [1, nt * MDU]])
                        eng.dma_start(dst, src)

            def dump(name_s, name_c, layer):
                if not debug:
                    return
                if name_s:
                    nc.scalar.dma_start(dbg[name_s].ap(), sp[:])
                nc.scalar.dma_start(dbg[name_c].ap(), ct[:])

            # ================= emission =================
            for _rep in range(reps):
                for layer in (1, 2, 3):
                    nc.vector.memset(v[:], 0.0)
                    for ch in range(NCH):
                        u0, tc_ = conv_chunk(layer, ch)
                        scan_rows(layer, u0, tc_)
                        if layer < 3 and (ch % 3 == 2 or ch == NCH - 1):
                            dup_group(layer, ch // 3)
                    if layer == 1:
                        if _rep == 0:
                            # I1M (conv1 input) is done with S2D's space now
                            sd_pad_rows(S2D, S2DF, BS2, T2D, CONV3)
                        dump("s1o", "c1o", 1)
                    elif layer == 2:
                        dump("s2o", "c2o", 2)
                    else:
                        dump(None, "c3o", 3)

            # ---- time-sum of s3 -> sbar [128, 80] (bf16 holds integers
            # <= 256 exactly, so the low-precision accumulate is exact) ----
            with nc.allow_low_precision(reason="spike counts <= 129 exact in bf16"):
                nc.vector.tensor_reduce(
                    sbar[:],
                    AP(sp[:].tensor, sp[:].offset + PMU,
                       [[SPF, 128], [SPR, 2], [1, M], [MDU, T]]),
                    axis=mybir.AxisListType.X, op=OP.add)
            if debug:
                nc.scalar.dma_start(dbg["sbo"].ap(), sbar[:])

            # ---- FC: y = (wf @ sbar)/T + bf ----
            # sbar [(c,bh), (b2,m)] -> scr[b' , c*40+m]  (b' = 2*b2 + bh)
            for bh in range(2):
                src = AP(sbar[:].tensor, sbar[:].offset + bh * 64 * FS,
                         [[FS, 64], [M, 2], [1, M]])
                dst = AP(scr[:].tensor, scr[:].offset + bh * C * M,
                         [[M, 64], [2 * C * M, 2], [1, M]])
                nc.scalar.dma_start(dst, src)
            with nc.allow_non_contiguous_dma(reason="tiny fc relayout"):
                for b in range(BL):
                    nc.scalar.dma_start(
                        AP(fcr[:].tensor, fcr[:].offset + b * 32,
                           [[32 * BL, 80], [1, 32]]),
                        AP(scr[:].tensor, scr[:].offset + b * C * M,
                           [[1, 80], [80, 32]]))
            pf = pfc.tile([12, BL], F32)
            for k in range(32):
                nc.tensor.matmul(
                    pf[:, :], fct[0:80, k * 12:(k + 1) * 12],
                    AP(fcr[:].tensor, fcr[:].offset + k, [[32 * BL, 80], [32, BL]]),
                    start=(k == 0), stop=(k == 31))
            nc.scalar.activation(ysb[:], pf[:, :], AF.Identity,
                                 bias=bft[:, 0:1], scale=float(1.0 / T))
            nc.scalar.dma_start(AP(y_d.ap().tensor, 0, [[1, 12], [12, BL]]), ysb[:])

    return nc


# ---------------- host-side weight packing ----------------
def pack_inputs(x, w1, w2, w3, wf, bf):
    """Returns list of per-core input maps (bf16-packed)."""
    inv_tau = np.float32(INV_TAU)
    x = np.asarray(x, np.float32)
    w1 = np.asarray(w1, np.float32)
    w2 = np.asarray(w2, np.float32)
    w3 = np.asarray(w3, np.float32)
    wf = np.asarray(wf, np.float32)

    w1p = np.zeros((12, 128), np.float32)
    for i in range(4):
        for jj in range(3):
            w1p[4 * jj + i, 0:64] = w1[:, 0, i, jj] * inv_tau
    w1p[:, 64:128] = w1p[:, 0:64]
    w1p = w1p.astype(ml_dtypes.bfloat16)

    def pack_w(w):
        wp = np.zeros((128, 768), np.float32)
        for mmo in range(6):
            tp, jj = mmo // 3, JJO[mmo % 3]
            for g in range(2):
                i = 2 * tp + g
                blk = w[:, :, i, jj].T * inv_tau   # [c_in, c_out]
                wp[g * 64:(g + 1) * 64, mmo * 128:mmo * 128 + 64] = blk
                wp[g * 64:(g + 1) * 64, mmo * 128 + 64:mmo * 128 + 128] = blk
        return wp.astype(ml_dtypes.bfloat16)

    w2p = pack_w(w2)
    w3p = pack_w(w3)
    fcp = np.zeros((80, 32 * 12), np.float32)
    for k in range(32):
        fcp[:, k * 12:(k + 1) * 12] = wf[:, 80 * k:80 * (k + 1)].T
    fcp = fcp.astype(ml_dtypes.bfloat16)
    bfv = np.asarray(bf, np.float32).reshape(12)

    maps = []
    for c in range(NCORES):
        xc = x[c * BL:(c + 1) * BL, 0]                    # [BL, T0, M]
        xp = np.ascontiguousarray(
            xc.transpose(1, 0, 2).reshape(T0, BL * M)).astype(ml_dtypes.bfloat16)
        maps.append({
            "x": xp, "w1p": w1p, "w2p": w2p, "w3p": w3p, "fcp": fcp, "bf": bfv,
            "zz": np.zeros(128, ml_dtypes.bfloat16),
        })
    return maps


_CACHED = {}


def get_nc(debug=False, reps=1, spike_pool=False):
    key = (bool(debug), reps, bool(spike_pool))
    if key not in _CACHED:
        nc = build_nc(debug=debug, reps=reps, spike_pool=spike_pool)
        merge_bbs(nc)
        _CACHED[key] = nc
    return _CACHED[key]


def make_runner(nc, in_maps):
    """Build the sharded PJRT callable once (mimics bass2jax.run_bass_via_pjrt)
    so repeated calls reuse the compiled executable for timing."""
    import jax
    from jax.sharding import Mesh, PartitionSpec
    from jax.experimental.shard_map import shard_map
    from concourse import bass2jax
    from concourse.bass2jax import _bass_exec_p, install_neuronx_cc_hook, partition_id_tensor

    install_neuronx_cc_hook()
    n_cores = len(in_maps)
    partition_name = nc.partition_id_tensor.name if nc.partition_id_tensor else None
    in_names, out_names, out_avals, zero_outs = [], [], [], []
    for alloc in nc.m.functions[0].allocations:
        if not isinstance(alloc, mybir.MemoryLocationSet):
            continue
        name = alloc.memorylocations[0].name
        if alloc.kind == "ExternalInput":
            if name != partition_name:
                in_names.append(name)
        elif alloc.kind == "ExternalOutput":
            out_names.append(name)
            shape = tuple(alloc.tensor_shape)
            dtype = mybir.dt.np(alloc.dtype)
            out_avals.append(jax.core.ShapedArray(shape, dtype))
            zero_outs.append(np.zeros(shape, dtype))
    n_params = len(in_names)
    n_outs = len(out_avals)
    in_names_all = in_names + out_names + ([partition_name] if partition_name else [])

    def _body(*args):
        operands = list(args)
        if partition_name is not None:
            operands.append(partition_id_tensor())
        outs = _bass_exec_p.bind(
            *operands,
            out_avals=tuple(out_avals),
            in_names=tuple(in_names_all),
            out_names=tuple(out_names),
            lowering_input_output_aliases=(),
            sim_require_finite=True,
            sim_require_nnan=True,
            nc=nc,
        )
        return tuple(outs)

    devices = jax.devices()[:n_cores]
    mesh = Mesh(np.asarray(devices), ("core",))
    donate = tuple(range(n_params, n_params + n_outs))
    sharded = jax.jit(
        shard_map(_body, mesh=mesh,
                  in_specs=(PartitionSpec("core"),) * (n_params + n_outs),
                  out_specs=(PartitionSpec("core"),) * n_outs,
                  check_rep=False),
        donate_argnums=donate, keep_unused=True)
    concat_in = [
        np.concatenate([np.asarray(in_maps[c][nm]) for c in range(n_cores)], axis=0)
        for nm in in_names
    ]

    def run():
        zeros = [np.zeros((n_cores * z.shape[0], *z.shape[1:]), z.dtype)
                 for z in zero_outs]
        out_arrs = sharded(*concat_in, *zeros)
        out_arrs = [np.asarray(a) for a in out_arrs]
        return [
            {nm: out_arrs[i].reshape(n_cores, *out_avals[i].shape)[c]
             for i, nm in enumerate(out_names)}
            for c in range(n_cores)
        ]

    return run


def kernel(x, w1, w2, w3, wf, bf):
    nc = get_nc(debug=False)
    in_maps = pack_inputs(np.asarray(x), np.asarray(w1), np.asarray(w2),
                          np.asarray(w3), np.asarray(wf), np.asarray(bf))
    res = run_bass_kernel_spmd(nc, in_maps, list(range(NCORES)))
    y = np.concatenate([res.results[c]["y"] for c in range(NCORES)], axis=0)
    return y.astype(np.float32)
